# revision 35
# baseline (speedup 1.0000x reference)
import os
import sys

import numpy as np

if "/opt/trn_rl_repo" not in sys.path:
    sys.path.insert(0, "/opt/trn_rl_repo")

B, T, N, F, H = 32, 64, 207, 64, 128
NCORES = 8
BPC = B // NCORES          # batches per core
R = BPC * N                # 828 rows per core per step
CH = R // 2                # 414 rows per matmul chunk (<=512 psum bank)
G3 = 3 * H

LAST_EXEC_NS = None
_NC = {}


def _build(t_steps=T, zero_bias=False):
    import concourse.bass as bass
    import concourse.mybir as mybir
    import concourse.tile as tile
    from concourse import bacc

    fp32 = mybir.dt.float32
    f32r = mybir.dt.float32r
    AF = mybir.ActivationFunctionType
    ALU = mybir.AluOpType

    nc = bacc.Bacc("TRN2", target_bir_lowering=False, debug=False)

    def mm(out, lhsT, rhs, start, stop):
        nc.tensor.matmul(
            out, lhsT.bitcast(f32r), rhs.bitcast(f32r), start=start, stop=stop
        )

    mdt = f32r if zero_bias else fp32
    x_in = nc.dram_tensor("x_in", (t_steps, F, R), mdt, kind="ExternalInput")
    w_ih0 = nc.dram_tensor("w_ih0", (F, G3), mdt, kind="ExternalInput")
    w_hh0 = nc.dram_tensor("w_hh0", (H, G3), mdt, kind="ExternalInput")
    w_ih1 = nc.dram_tensor("w_ih1", (H, G3), mdt, kind="ExternalInput")
    w_hh1 = nc.dram_tensor("w_hh1", (H, G3), mdt, kind="ExternalInput")
    biases = nc.dram_tensor("biases", (H, 8), fp32, kind="ExternalInput")
    ident = nc.dram_tensor("ident", (H, H), mdt, kind="ExternalInput")
    out_fm = nc.dram_tensor("out_fm", (t_steps, H, R), mdt, kind="ExternalOutput")
    h0f = nc.dram_tensor("h0f", (H, R), mdt, kind="ExternalOutput")
    # zero_bias: h1 final state == out_fm[T-1], recovered host-side
    h1f = None if zero_bias else nc.dram_tensor(
        "h1f", (H, R), mdt, kind="ExternalOutput")

    import contextlib

    with tile.TileContext(nc) as tc:
        with contextlib.ExitStack() as st:
            cpool = st.enter_context(tc.tile_pool(name="const", bufs=1))
            xpool = st.enter_context(tc.tile_pool(name="xio", bufs=3))
            wpool = st.enter_context(tc.tile_pool(name="work", bufs=4))
            if zero_bias:
                przp = st.enter_context(
                    tc.tile_pool(name="psum_rz", bufs=2, space=bass.MemorySpace.PSUM))
                pnp = st.enter_context(
                    tc.tile_pool(name="psum_n", bufs=2, space=bass.MemorySpace.PSUM))
            else:
                ppool = st.enter_context(
                    tc.tile_pool(name="psum", bufs=2, space=bass.MemorySpace.PSUM))
            xt0 = None
            if zero_bias:
                xt0 = xpool.tile((F, R), mdt, name="xt")
                nc.sync.dma_start(xt0[:], x_in[0, :, :])
            wg = {}
            for nm, dram, k in (
                ("ih0", w_ih0, F),
                ("ih1", w_ih1, H),
                ("hh0", w_hh0, H),
                ("hh1", w_hh1, H),
            ):
                for g in range(3):
                    wt = cpool.tile((k, H), mdt, name=f"w_{nm}_{g}")
                    nc.sync.dma_start(wt[:], dram[:, g * H:(g + 1) * H])
                    wg[nm, g] = wt
            it = cpool.tile((H, H), mdt, name="ident_sb")
            nc.sync.dma_start(it[:], ident[:])
            bt = cpool.tile((H, 8), fp32, name="bt")
            nc.sync.dma_start(bt[:], biases[:])

            if zero_bias:
                h0b = [cpool.tile((H, R), mdt, name=f"h0_{p}") for p in range(2)]
                h1b = [cpool.tile((H, R), mdt, name=f"h1_{p}") for p in range(2)]

                def cell0(src, hnew, ih):
                    # t=0: h=0, so r is unused (r*h_n=0) — only z and i_n needed
                    for c in range(2):
                        sl = slice(c * CH, (c + 1) * CH)
                        Prz = przp.tile((H, 2, 512), fp32, name="Prz")
                        Pn = pnp.tile((H, 2, 512), fp32, name="Pn")
                        mm(Prz[:, 1, 0:CH], wg[ih, 1][:], src[:, sl], True, True)
                        mm(Pn[:, 0, 0:CH], wg[ih, 2][:], src[:, sl], True, True)
                        z0 = wpool.tile((H, CH), fp32, name="z0")
                        nc.scalar.activation(z0[:], Prz[:, 1, 0:CH], AF.Sigmoid)
                        n = wpool.tile((H, CH), fp32, name="n")
                        nc.scalar.activation(n[:], Pn[:, 0, 0:CH], AF.Tanh)
                        zc = wpool.tile((H, CH), fp32, name="zc")
                        nc.vector.tensor_scalar(
                            zc[:], z0[:], -1.0, 1.0, ALU.mult, ALU.add)
                        nc.gpsimd.tensor_mul(hnew[:, sl], zc[:], n[:])

                def cell(src, hold, hnew, ih, hh):
                    half = []
                    for c in range(2):
                        sl = slice(c * CH, (c + 1) * CH)
                        Prz = przp.tile((H, 2, 512), fp32, name="Prz")
                        Pn = pnp.tile((H, 2, 512), fp32, name="Pn")
                        mm(Prz[:, 0, 0:CH], wg[ih, 0][:], src[:, sl], True, False)
                        mm(Prz[:, 0, 0:CH], wg[hh, 0][:], hold[:, sl], False, True)
                        mm(Prz[:, 1, 0:CH], wg[ih, 1][:], src[:, sl], True, False)
                        mm(Prz[:, 1, 0:CH], wg[hh, 1][:], hold[:, sl], False, True)
                        mm(Pn[:, 0, 0:CH], wg[ih, 2][:], src[:, sl], True, False)
                        mm(Pn[:, 1, 0:CH], wg[hh, 2][:], hold[:, sl], True, True)
                        rz = wpool.tile((H, 2, CH), fp32, name="rz")
                        nc.scalar.activation(rz[:], Prz[:, :, 0:CH], AF.Sigmoid)
                        half.append((sl, Pn, rz))
                    half2 = []
                    for sl, Pn, rz in half:
                        tt = wpool.tile((H, CH), mdt, name="tt")
                        nc.vector.tensor_mul(tt[:], Pn[:, 1, 0:CH], rz[:, 0, :])
                        mm(Pn[:, 0, 0:CH], it[:], tt[:], False, True)
                        n = wpool.tile((H, CH), fp32, name="n")
                        nc.scalar.activation(n[:], Pn[:, 0, 0:CH], AF.Tanh)
                        a = wpool.tile((H, CH), fp32, name="a")
                        nc.gpsimd.tensor_mul(a[:], rz[:, 1, :], hold[:, sl])
                        half2.append((sl, rz, n, a))
                    for sl, rz, n, a in half2:
                        zc = wpool.tile((H, CH), fp32, name="zc")
                        nc.vector.tensor_scalar(
                            zc[:], rz[:, 1, :], -1.0, 1.0, ALU.mult, ALU.add)
                        b2 = wpool.tile((H, CH), fp32, name="b2")
                        nc.gpsimd.tensor_mul(b2[:], zc[:], n[:])
                        nc.gpsimd.tensor_add(hnew[:, sl], a[:], b2[:])

                cell0(xt0, h0b[1], "ih0")
                for t in range(t_steps):
                    if t + 1 < t_steps:
                        xt = xpool.tile((F, R), mdt, name="xt")
                        nc.sync.dma_start(xt[:], x_in[t + 1, :, :])
                        cell(xt, h0b[(t + 1) % 2], h0b[t % 2], "ih0", "hh0")
                    else:
                        nc.sync.dma_start(h0f[:], h0b[t_steps % 2][:])
                    if t == 0:
                        cell0(h0b[1], h1b[1], "ih1")
                    else:
                        cell(h0b[(t + 1) % 2], h1b[t % 2], h1b[(t + 1) % 2],
                             "ih1", "hh1")
                    nc.sync.dma_start(out_fm[t, :, :], h1b[(t + 1) % 2][:])
            else:
                h = [cpool.tile((H, R), fp32, name=f"h{l}") for l in range(2)]
                nc.vector.memset(h[0][:], 0.0)
                nc.vector.memset(h[1][:], 0.0)

                for t in range(t_steps):
                    xt = xpool.tile((F, R), fp32, name="xt")
                    nc.sync.dma_start(xt[:], x_in[t, :, :])
                    for l in range(2):
                        ih, hh = ("ih0", "hh0") if l == 0 else ("ih1", "hh1")
                        src = xt if l == 0 else h[0]
                        hl = h[l]
                        bo = 4 * l
                        for c in range(2):
                            sl = slice(c * CH, (c + 1) * CH)
                            P = ppool.tile((H, 4, 512), fp32, name="P")
                            mm(P[:, 0, 0:CH], wg[ih, 0][:], src[:, sl], True, False)
                            mm(P[:, 0, 0:CH], wg[hh, 0][:], hl[:, sl], False, True)
                            mm(P[:, 1, 0:CH], wg[ih, 1][:], src[:, sl], True, False)
                            mm(P[:, 1, 0:CH], wg[hh, 1][:], hl[:, sl], False, True)
                            mm(P[:, 2, 0:CH], wg[ih, 2][:], src[:, sl], True, False)
                            mm(P[:, 3, 0:CH], wg[hh, 2][:], hl[:, sl], True, True)
                            r = wpool.tile((H, CH), fp32, name="r")
                            z = wpool.tile((H, CH), fp32, name="z")
                            nc.scalar.activation(r[:], P[:, 0, 0:CH], AF.Sigmoid, bias=bt[:, bo:bo + 1])
                            nc.scalar.activation(z[:], P[:, 1, 0:CH], AF.Sigmoid, bias=bt[:, bo + 1:bo + 2])
                            # tt = (h_n + b_hhn) * r
                            tt = wpool.tile((H, CH), fp32, name="tt")
                            nc.vector.scalar_tensor_tensor(
                                tt[:], P[:, 3, 0:CH], bt[:, bo + 3:bo + 4], r[:],
                                ALU.add, ALU.mult,
                            )
                            # P_n += tt  (identity matmul accumulate)
                            mm(P[:, 2, 0:CH], it[:], tt[:], False, True)
                            n = wpool.tile((H, CH), fp32, name="n")
                            nc.scalar.activation(n[:], P[:, 2, 0:CH], AF.Tanh, bias=bt[:, bo + 2:bo + 3])
                            # h' = n + z * (h - n)
                            d = wpool.tile((H, CH), fp32, name="d")
                            nc.gpsimd.tensor_sub(d[:], hl[:, sl], n[:])
                            e = wpool.tile((H, CH), fp32, name="e")
                            nc.vector.tensor_mul(e[:], z[:], d[:])
                            nc.vector.tensor_add(hl[:, sl], e[:], n[:])
                    nc.sync.dma_start(out_fm[t, :, :], h[1][:])
                nc.sync.dma_start(h0f[:], h[0][:])
                nc.sync.dma_start(h1f[:], h[1][:])

    nc.compile()
    return nc


def _get_nc(zero_bias):
    if zero_bias not in _NC:
        _NC[zero_bias] = _build(zero_bias=zero_bias)
    return _NC[zero_bias]


def _bias_pack(b_ih0, b_hh0, b_ih1, b_hh1):
    return np.stack(
        [
            b_ih0[0:H] + b_hh0[0:H],
            b_ih0[H:2 * H] + b_hh0[H:2 * H],
            b_ih0[2 * H:3 * H],
            b_hh0[2 * H:3 * H],
            b_ih1[0:H] + b_hh1[0:H],
            b_ih1[H:2 * H] + b_hh1[H:2 * H],
            b_ih1[2 * H:3 * H],
            b_hh1[2 * H:3 * H],
        ],
        axis=1,
    ).astype(np.float32)


def _f32r_round(a):
    u = np.ascontiguousarray(a).view(np.uint32)
    u = (u.astype(np.uint64) + 0x7FF + ((u >> 12) & 1)) & 0xFFFFF000
    return u.astype(np.uint32).view(np.float32)


def kernel(x, W_ih0, W_hh0, b_ih0, b_hh0, W_ih1, W_hh1, b_ih1, b_hh1):
    global LAST_EXEC_NS
    from concourse import bass_utils

    x = np.asarray(x, np.float32)
    W_ih0 = np.ascontiguousarray(np.asarray(W_ih0, np.float32))
    W_hh0 = np.ascontiguousarray(np.asarray(W_hh0, np.float32))
    W_ih1 = np.ascontiguousarray(np.asarray(W_ih1, np.float32))
    W_hh1 = np.ascontiguousarray(np.asarray(W_hh1, np.float32))
    bias = _bias_pack(
        np.asarray(b_ih0, np.float32), np.asarray(b_hh0, np.float32),
        np.asarray(b_ih1, np.float32), np.asarray(b_hh1, np.float32),
    )
    ident = np.eye(H, dtype=np.float32)

    zb = not bias.any()
    nc = _get_nc(zero_bias=zb)
    if zb:
        x = _f32r_round(x)
        W_ih0 = _f32r_round(W_ih0)
        W_hh0 = _f32r_round(W_hh0)
        W_ih1 = _f32r_round(W_ih1)
        W_hh1 = _f32r_round(W_hh1)
    in_maps = []
    for c in range(NCORES):
        xc = x[c * BPC:(c + 1) * BPC]  # [BPC, T, N, F]
        x_fm = np.ascontiguousarray(
            np.transpose(xc, (1, 3, 0, 2)).reshape(T, F, R)
        )
        in_maps.append({
            "x_in": x_fm,
            "w_ih0": W_ih0, "w_hh0": W_hh0,
            "w_ih1": W_ih1, "w_hh1": W_hh1,
            "biases": bias, "ident": ident,
        })

    trace = os.environ.get("GRU_TRACE", "0") == "1"
    kw = {}
    if trace:
        kw["trace"] = True
        td = os.environ.get("GRU_TRACE_DIR")
        if td:
            os.makedirs(td, exist_ok=True)
            kw["tmpdir"] = td
    try:
        res = bass_utils.run_bass_kernel_spmd(
            nc, in_maps, core_ids=list(range(NCORES)), **kw
        )
    except Exception:
        if not kw:
            raise
        res = bass_utils.run_bass_kernel_spmd(
            nc, in_maps, core_ids=list(range(NCORES))
        )
    LAST_EXEC_NS = res.exec_time_ns

    out = np.empty((B, T, N, H), np.float32)
    h0 = np.empty((B, N, H), np.float32)
    h1 = np.empty((B, N, H), np.float32)
    for c in range(NCORES):
        rc = res.results[c]
        o = np.asarray(rc["out_fm"]).reshape(T, H, BPC, N)
        out[c * BPC:(c + 1) * BPC] = np.transpose(o, (2, 0, 3, 1))
        h0[c * BPC:(c + 1) * BPC] = np.transpose(
            np.asarray(rc["h0f"]).reshape(H, BPC, N), (1, 2, 0))
        if zb:
            h1[c * BPC:(c + 1) * BPC] = out[c * BPC:(c + 1) * BPC, -1]
        else:
            h1[c * BPC:(c + 1) * BPC] = np.transpose(
                np.asarray(rc["h1f"]).reshape(H, BPC, N), (1, 2, 0))
    return out, h0, h1
